# revision 1
# baseline (speedup 1.0000x reference)
"""MetapathAttentionLayer Trainium2 kernel (v2).

Math (per node n):
    scores[n, m] = sum_d x[m, n, d] * W[d, m]
    att = softmax(relu(scores), axis=m)      (8 metapaths)
    out[n, :] = elu(sum_m att[n, m] * x[m, n, :])

Strategy: shard nodes across 8 cores (data parallel). Per core 12544 nodes
(12500 + pad). SBUF layout "B": partition p = (m, r) with m = metapath (8),
r = node-residue (16); node n = r*784 + c for chunk-column c in [0, 784).

Per group of 56 chunk-columns (896 nodes):
  - scores: DVE bf16 multiply (2x mode) + log-tree halving adds over d
    (upper levels DVE 2x, lower levels GPSIMD), exp on ACT (one act table),
    e1 = max(e,1) on GPSIMD.
  - softmax over m lives on partition groups: a single PE matmul with a
    delta(r==r') replication mask sums e1 over m AND replicates Z to all
    128 partitions at once; 1/Z via DVE fast reciprocal (PSUM read);
    att = e1 * invZ on GPSIMD.
  - pooling: GPSIMD local_scatter builds sparse A[128, 56*16] with
    A[(m,r), c*16+i] = att * delta(r==i); one PE matmul per chunk
    (lhsT = X-chunk [128, 128d], rhs = A-slice [128, 16]) computes the full
    weighted sum over m AND selects nodes: U^T[d, 16] in PSUM, 16 cycles.
  - elu(u) = relu(u) + exp(min(u,0)) - 1: Relu/Exp passes on ACT (same
    activation table as exp), add on GPSIMD, transpose back to [node, d]
    via PE transpose-matmuls, final ACT Copy applies the -1 bias.
"""

import numpy as np
import ml_dtypes

import concourse.bass as bass
import concourse.tile as tile
from concourse import bacc, mybir, library_config
import concourse.bass_utils as bass_utils

F32 = mybir.dt.float32
BF16 = mybir.dt.bfloat16
I16 = mybir.dt.int16
ALU = mybir.AluOpType
ACTF = mybir.ActivationFunctionType

NMETA = 8
N = 100000
D = 128
NCORES = 8
NC_RAW = N // NCORES          # 12500 nodes per core
R = 16                        # node-residue groups on partitions
NB = 784                      # chunk-columns per residue: 16*784 = 12544
NC_PAD = R * NB               # padded nodes per core
CT = 56                       # chunk-columns per group
NG = NB // CT                 # 14 groups per core
GN = R * CT                   # nodes per group (896)
BLK = 8                       # chunk-columns per transpose window (128 nodes)
NBLK = CT // BLK              # transpose windows per group (7)


def kernel_body(tc, out_d, x_d, wb_d, mask_d, rep_d, sidx_d, eye_d, reps=1,
                out_q="sp", out_delay=7, bufs_elu=3, bufs_tree=3,
                bufs_small=8, bufs_a=4, bufs_osb=5, bufs_x=5, bufs_tree2=4,
                bufs_p=2, bufs_psr=1, variant="a", sizes=None):
    nc = tc.nc
    if sizes is None:
        sizes = [16, 40] + [56] * 13
    assert sum(sizes) == NB and all(s % 2 == 0 for s in sizes)
    offs = [0]
    for s_ in sizes:
        offs.append(offs[-1] + s_)
    NGv = len(sizes)
    from contextlib import ExitStack
    with ExitStack() as ctx:
        const = ctx.enter_context(tc.tile_pool(name="const", bufs=1))
        xpool = ctx.enter_context(tc.tile_pool(name="x", bufs=bufs_x))
        ppool = ctx.enter_context(tc.tile_pool(name="p", bufs=bufs_p))
        tpool = ctx.enter_context(tc.tile_pool(name="tree", bufs=bufs_tree))
        tpool2 = ctx.enter_context(tc.tile_pool(name="tree2", bufs=bufs_tree2))
        spool = ctx.enter_context(tc.tile_pool(name="small", bufs=bufs_small))
        apool = ctx.enter_context(tc.tile_pool(name="amat", bufs=bufs_a))
        epool = ctx.enter_context(tc.tile_pool(name="elu", bufs=bufs_elu))
        opool = ctx.enter_context(tc.tile_pool(name="osb", bufs=bufs_osb))
        psU = ctx.enter_context(tc.tile_pool(name="psU", bufs=2, space="PSUM"))
        psY = ctx.enter_context(tc.tile_pool(name="psY", bufs=1, space="PSUM"))
        psZ = ctx.enter_context(tc.tile_pool(name="psZ", bufs=2, space="PSUM"))
        psR = ctx.enter_context(tc.tile_pool(name="psR", bufs=bufs_psr, space="PSUM"))

        cst = {}

        def load_consts():
            wb_t = const.tile([128, D], BF16)
            nc.sync.dma_start(wb_t[:], wb_d[:])
            rep_t = const.tile([128, 128], BF16)
            nc.sync.dma_start(rep_t[:], rep_d[:])
            sidx_t = const.tile([128, CT], I16)
            nc.sync.dma_start(sidx_t[:], sidx_d[:])
            eye_t = const.tile([128, 128], BF16)
            nc.sync.dma_start(eye_t[:], eye_d[:])
            cst.update(wb=wb_t, repmat=rep_t, sidx=sidx_t, eye=eye_t)
            nc.gpsimd.load_library(library_config.local_scatter)

        # dram views for output addressing: [r, c, d]
        out_v = out_d[:].rearrange("(r c) d -> r c d", r=R)

        for _rep in range(reps):
            # Software-pipelined emission; see stage schedule in module doc.
            dma_eng = nc.scalar if out_q == "act" else nc.sync
            st = [dict() for _ in range(NGv)]
            pending = []

            def load(g):
                ct = sizes[g]
                X = xpool.tile([128, ct * D], BF16, tag="X")
                nc.sync.dma_start(
                    X[:], x_d[:, offs[g] * D:(offs[g] + ct) * D])
                st[g]["Xv"] = X[:].rearrange("p (c d) -> p c d", c=ct)

            def dve_scores(g):
                ct = sizes[g]
                Xv = st[g]["Xv"]
                P = ppool.tile([128, ct * D], BF16, tag="P")
                Pv = P[:].rearrange("p (c d) -> p c d", c=ct)
                nc.vector.tensor_tensor(
                    out=Pv, in0=Xv,
                    in1=cst["wb"][:].unsqueeze(1).broadcast_to([128, ct, D]),
                    op=ALU.mult)
                P1 = tpool.tile([128, ct * 64], BF16, tag="P1")
                P1v = P1[:].rearrange("p (c d) -> p c d", c=ct)
                nc.vector.tensor_tensor(
                    out=P1v, in0=Pv[:, :, 0:64], in1=Pv[:, :, 64:128],
                    op=ALU.add)
                P2 = tpool.tile([128, ct * 32], BF16, tag="P2")
                P2v = P2[:].rearrange("p (c d) -> p c d", c=ct)
                nc.vector.tensor_tensor(
                    out=P2v, in0=P1v[:, :, 0:32], in1=P1v[:, :, 32:64],
                    op=ALU.add)
                P3 = tpool2.tile([128, ct * 16], BF16, tag="P3")
                P3v = P3[:].rearrange("p (c d) -> p c d", c=ct)
                nc.vector.tensor_tensor(
                    out=P3v, in0=P2v[:, :, 0:16], in1=P2v[:, :, 16:32],
                    op=ALU.add)
                st[g]["P3v"] = P3v

            def gp_tree_softmax(g):
                ct = sizes[g]
                P3v = st[g].pop("P3v")
                P4 = tpool2.tile([128, ct * 8], BF16, tag="P4")
                P4v = P4[:].rearrange("p (c d) -> p c d", c=ct)
                nc.gpsimd.tensor_tensor(
                    out=P4v, in0=P3v[:, :, 0:8], in1=P3v[:, :, 8:16],
                    op=ALU.add)
                P5 = tpool2.tile([128, ct * 4], BF16, tag="P5")
                P5v = P5[:].rearrange("p (c d) -> p c d", c=ct)
                nc.gpsimd.tensor_tensor(
                    out=P5v, in0=P4v[:, :, 0:4], in1=P4v[:, :, 4:8],
                    op=ALU.add)
                P6 = tpool2.tile([128, ct * 2], BF16, tag="P6")
                P6v = P6[:].rearrange("p (c d) -> p c d", c=ct)
                nc.gpsimd.tensor_tensor(
                    out=P6v, in0=P5v[:, :, 0:2], in1=P5v[:, :, 2:4],
                    op=ALU.add)
                scores = spool.tile([128, ct], F32, tag="scores")
                nc.gpsimd.tensor_tensor(
                    out=scores[:].unsqueeze(2),
                    in0=P6v[:, :, 0:1], in1=P6v[:, :, 1:2],
                    op=ALU.add)
                st[g]["scores"] = scores

            def act_exp(g):
                ct = sizes[g]
                e = spool.tile([128, ct], F32, tag="e")
                nc.scalar.activation(e[:], st[g].pop("scores")[:], ACTF.Exp)
                st[g]["e"] = e

            def gp_e1(g):
                ct = sizes[g]
                e1 = spool.tile([128, ct], BF16, tag="e1")
                nc.gpsimd.tensor_scalar(e1[:], st[g].pop("e")[:], 1.0, None,
                                        ALU.max)
                st[g]["e1"] = e1

            def pe_Z(g):
                ct = sizes[g]
                Z = psZ.tile([128, ct], F32, tag="Z")
                nc.tensor.matmul(out=Z[:], lhsT=cst["repmat"][:], rhs=st[g]["e1"][:],
                                 start=True, stop=True)
                st[g]["Z"] = Z

            def act_zcopy(g):
                ct = sizes[g]
                Zs = spool.tile([128, ct], F32, tag="Zs")
                nc.scalar.activation(Zs[:], st[g].pop("Z")[:], ACTF.Copy)
                st[g]["Zs"] = Zs

            def dve_recip(g):
                ct = sizes[g]
                key = "Zs" if "Zs" in st[g] else "Z"
                inv = spool.tile([128, ct], F32, tag="inv")
                nc.vector.reciprocal_approx_fast(out=inv[:],
                                                 in_=st[g].pop(key)[:])
                st[g]["inv"] = inv

            def gp_att_scatter(g):
                ct = sizes[g]
                att = spool.tile([128, ct], BF16, tag="att")
                nc.gpsimd.tensor_tensor(out=att[:], in0=st[g].pop("e1")[:],
                                        in1=st[g].pop("inv")[:],
                                        op=ALU.mult)
                A = apool.tile([128, ct * R], BF16, tag="A")
                nc.gpsimd.local_scatter(A[:], att[:], cst["sidx"][:, 0:ct],
                                        channels=128, num_elems=ct * R,
                                        num_idxs=ct)
                st[g]["A"] = A

            def pe_pool(g):
                ct = sizes[g]
                Xv = st[g].pop("Xv")
                A = st[g].pop("A")
                U = psU.tile([128, ct * R], F32, tag="U")
                for c in range(ct):
                    nc.tensor.matmul(
                        out=U[:, c * R:(c + 1) * R],
                        lhsT=Xv[:, c, :],
                        rhs=A[:, c * R:(c + 1) * R],
                        start=True, stop=True)
                st[g]["U"] = U

            def act_elu(g):
                ct = sizes[g]
                U = st[g].pop("U")
                b = epool.tile([128, ct * R], BF16, tag="b")
                nc.scalar.activation(b[:], U[:], ACTF.Relu, scale=-1.0)
                c2 = epool.tile([128, ct * R], BF16, tag="c2")
                nc.scalar.activation(c2[:], b[:], ACTF.Exp, scale=-1.0)
                a = epool.tile([128, ct * R], BF16, tag="a")
                nc.scalar.activation(a[:], U[:], ACTF.Relu)
                st[g]["a"] = a
                st[g]["c2"] = c2

            def gp_combine(g):
                ct = sizes[g]
                y = epool.tile([128, ct * R], BF16, tag="y")
                nc.gpsimd.tensor_tensor(out=y[:], in0=st[g].pop("a")[:],
                                        in1=st[g].pop("c2")[:], op=ALU.add)
                st[g]["y"] = y

            def pe_transpose(g):
                ct = sizes[g]
                yT = psY.tile([ct, R * D], BF16, tag="yT")
                yv = st[g].pop("y")[:].rearrange("p (c r) -> p r c", r=R)
                for r in range(R):
                    nc.tensor.matmul(
                        out=yT[:, r * D:(r + 1) * D],
                        lhsT=yv[:, r, :],
                        rhs=cst["eye"][:],
                        is_transpose=True,
                        start=True, stop=True)
                st[g]["yT"] = yT

            def act_final(g):
                ct = sizes[g]
                osb = opool.tile([ct, R * D], BF16, tag="osb")
                nc.scalar.activation(osb[:], st[g].pop("yT")[:], ACTF.Copy,
                                     bias=-1.0)
                dst = out_v[:, offs[g]:offs[g] + ct, :].rearrange(
                    "r c d -> c r d")
                pending.append((dst, osb[:]))

            def ok(g):
                return g is not None and 0 <= g < NGv

            def emit_variant_a(it):
                if ok(it):
                    dve_scores(it)
                if ok(it - 1):
                    dve_recip(it - 1)
                if ok(it - 3):
                    gp_combine(it - 3)
                    pe_transpose(it - 3)
                if ok(it - 2):
                    act_elu(it - 2)
                if ok(it - 3):
                    act_final(it - 3)
                if ok(it):
                    gp_tree_softmax(it)
                    act_exp(it)
                    gp_e1(it)
                    pe_Z(it)
                if ok(it - 1):
                    gp_att_scatter(it - 1)
                    pe_pool(it - 1)

            def emit_variant_c(it):
                if ok(it):
                    dve_scores(it)
                if ok(it - 2):
                    dve_recip(it - 2)
                if ok(it - 5):
                    gp_combine(it - 5)
                    pe_transpose(it - 5)
                if ok(it - 4):
                    act_elu(it - 4)
                if ok(it - 5):
                    act_final(it - 5)
                if ok(it - 1):
                    act_exp(it - 1)
                if ok(it - 3):
                    act_invcopy(it - 3)
                if ok(it - 1):
                    pe_Z(it - 1)
                    act_zcopy(it - 1)
                if ok(it - 2):
                    pe_invrep(it - 2)
                if ok(it):
                    gp_tree_softmax(it)
                if ok(it - 3):
                    gp_att_scatter(it - 3)
                    pe_pool(it - 3)

            def emit_variant_d(it):
                if ok(it):
                    dve_scores(it)
                if ok(it - 2):
                    dve_recip(it - 2)
                if ok(it - 4):
                    gp_combine(it - 4)
                    pe_transpose(it - 4)
                if ok(it - 3):
                    act_elu(it - 3)
                if ok(it - 4):
                    act_final(it - 4)
                if ok(it - 1):
                    act_exp(it - 1)
                    pe_Z(it - 1)
                if ok(it - 2):
                    pe_invrep(it - 2)
                    act_invcopy(it - 2)
                if ok(it):
                    gp_tree_softmax(it)
                if ok(it - 2):
                    gp_att_scatter(it - 2)
                    pe_pool(it - 2)
                if ok(it - 1):
                    act_zcopy(it - 1)

            def emit_variant_e(it):
                if ok(it):
                    dve_scores(it)
                if ok(it - 3):
                    dve_recip(it - 3)
                if ok(it - 5):
                    gp_combine(it - 5)
                    pe_transpose(it - 5)
                if ok(it - 4):
                    act_elu(it - 4)
                if ok(it - 5):
                    act_final(it - 5)
                if ok(it - 1):
                    act_zcopy(it - 1)
                if ok(it):
                    gp_tree_softmax(it)
                    act_exp(it)
                    pe_Z(it)
                if ok(it - 3):
                    pe_invrep(it - 3)
                    act_invcopy(it - 3)
                    gp_att_scatter(it - 3)
                    pe_pool(it - 3)

            def emit_variant_z(it):
                if ok(it):
                    dve_scores(it)
                if ok(it - 2):
                    dve_recip(it - 2)
                if ok(it - 4):
                    gp_combine(it - 4)
                    pe_transpose(it - 4)
                if ok(it - 3):
                    act_elu(it - 3)
                if ok(it - 4):
                    act_final(it - 4)
                if ok(it - 1):
                    act_zcopy(it - 1)
                if ok(it):
                    gp_tree_softmax(it)
                    act_exp(it)
                    gp_e1(it)
                    pe_Z(it)
                if ok(it - 2):
                    gp_att_scatter(it - 2)
                    pe_pool(it - 2)

            emit = {"a": emit_variant_a, "c": emit_variant_c,
                    "d": emit_variant_d, "e": emit_variant_e,
                    "z": emit_variant_z}[variant]


            load(0)
            load_consts()
            for it in range(NGv + 7):
                if 0 < it < NGv:
                    load(it)
                if pending and it >= out_delay:
                    dst, src_ = pending.pop(0)
                    dma_eng.dma_start(dst, src_)
                emit(it)
            for dst, src_ in pending:
                dma_eng.dma_start(dst, src_)


def host_inputs(x_np, w_np):
    """Build per-core input maps from full fp32 inputs."""
    in_maps = []
    w_bf = w_np.astype(ml_dtypes.bfloat16)          # [D, NMETA]
    # wb[(m,r), d] = W[d, m]
    wb = np.ascontiguousarray(
        np.repeat(w_bf.T, R, axis=0))               # [128, D]
    mask = np.zeros((128, R), dtype=ml_dtypes.bfloat16)
    for m in range(NMETA):
        for r in range(R):
            mask[m * R + r, r] = 1.0
    rep = np.zeros((128, 128), dtype=ml_dtypes.bfloat16)
    for p in range(128):
        for m2 in range(NMETA):
            rep[p, m2 * R + (p % R)] = 1.0
    sidx = np.zeros((128, CT), dtype=np.int16)
    for p in range(128):
        r = p % R
        for c in range(CT):
            sidx[p, c] = c * R + r
    eye = np.eye(128, dtype=ml_dtypes.bfloat16)

    nc_raw = x_np.shape[1] // NCORES
    for core in range(NCORES):
        xs = x_np[:, core * nc_raw:(core + 1) * nc_raw, :]
        xp = np.zeros((NMETA, NC_PAD, D), dtype=ml_dtypes.bfloat16)
        xp[:, :nc_raw, :] = xs.astype(ml_dtypes.bfloat16)
        # xb[(m, r), (c, d)] = x[m, r*NB + c, d]
        xb = np.ascontiguousarray(
            xp.reshape(NMETA, R, NB * D).reshape(128, NB * D))
        in_maps.append({"x": xb, "wb": wb, "mask": mask, "rep": rep,
                        "sidx": sidx, "eye": eye})
    return in_maps


_CACHE = {}


def build(reps=1, **kw):
    key = (reps, tuple(sorted((k, tuple(v) if isinstance(v, list) else v)
                              for k, v in kw.items())))
    if key in _CACHE:
        return _CACHE[key]
    nc = bacc.Bacc("TRN2", target_bir_lowering=False, debug=False,
                   num_devices=NCORES)
    x = nc.dram_tensor("x", [128, NB * D], BF16, kind="ExternalInput").ap()
    wb = nc.dram_tensor("wb", [128, D], BF16, kind="ExternalInput").ap()
    mask = nc.dram_tensor("mask", [128, R], BF16, kind="ExternalInput").ap()
    rep = nc.dram_tensor("rep", [128, 128], BF16, kind="ExternalInput").ap()
    sidx = nc.dram_tensor("sidx", [128, CT], I16, kind="ExternalInput").ap()
    eye = nc.dram_tensor("eye", [128, 128], BF16, kind="ExternalInput").ap()
    out = nc.dram_tensor("out", [NC_PAD, D], BF16, kind="ExternalOutput").ap()
    with tile.TileContext(nc) as tc:
        kernel_body(tc, out, x, wb, mask, rep, sidx, eye, reps=reps, **kw)
    nc.compile()
    _CACHE[key] = nc
    return nc


def run(input, W, trace=False, **trace_kwargs):
    x_np = np.asarray(input, dtype=np.float32)
    w_np = np.asarray(W, dtype=np.float32)
    nc = build()
    in_maps = host_inputs(x_np, w_np)
    res = bass_utils.run_bass_kernel_spmd(
        nc, in_maps, core_ids=list(range(NCORES)), trace=trace, **trace_kwargs)
    nc_raw = x_np.shape[1] // NCORES
    full = np.concatenate(
        [res.results[c]["out"][:nc_raw] for c in range(NCORES)],
        axis=0).astype(np.float32)
    return full, res


def kernel(input, W):
    out, _ = run(input, W, trace=False)
    return out

